# revision 34
# baseline (speedup 1.0000x reference)
"""Trainium2 Bass kernel for a recurrent adaptive-LIF SNN.

Network (per reference):
    B=1024, T=100, n_in=120, h1=512, h2=256, n_out=35
    per step t:
        cur1 = x_t @ W1.T + s1 @ Wrec.T
        a1' = rho1*a1 + (1-rho1)*s1 ; v1' = alpha1*v1*(1-s1) + (1-alpha1)*cur1
        s1' = (v1' - (1 + beta_a1*a1') > 0)
        cur2 = s1' @ W2.T ; same LIF for layer 2
        vo' = beta_out*vo + (1-beta_out)*(s2' @ W3.T) ; out = mean_t vo

Sharding: data-parallel over batch across 8 cores (BC=128 per core),
weights replicated; the sequential T loop is local per core.

Reformulation (shifted potential): d := LAM*(v - 1 - beta_a*a) lives in
PSUM (LAM=32 keeps fp8 stationaries in normal range).  With C =
LAM*(alpha-1) and A = -(alpha+cb), cb = beta_a*(1-rho):
    spike       s' = (d + C > 0)
    reset carry g' = min(d + C, 0)        [ = LAM * v'(1-s') - shift ]
    d(t+1) = LAM(1-a)*W@in + LAM*A*s' + a*g' + C-as-op-bias
The identity v'(1-s') <-> min(d,0) makes both nonlinear updates UNARY in
d, so no tensor-tensor products are needed.  The residual adaptation
couplings ((alpha-rho)*cb*u and alpha*cb*u*s', coefficients 7.5e-4 /
7.1e-3) are below the fp8 weight-quantization noise and are dropped;
they are exactly zero whenever the layer is silent.
Output integrator is collapsed into per-step weights:
    out = sum_t w_t * W3 @ s2(t),  w_t = (1-beta_out^(T-t))/T,
one DoubleRow matmul per step accumulating into PSUM.

Scheduling: the tile framework serializes readers of the same PSUM tile
(across engines, in emission order), so d1 is split into two PSUM
tiles.  Chunks 0-2 run a DVE chain: min-carry from PSUM, then the spike
derived from the carry in SBUF at 2x.  Chunk 3 runs on ACT: a +-1 Sign
spike directly from PSUM first, then the flipped relu carry as the
serialized second reader (its consumer, the next step's diagonal, has a
full cycle of slack).  Only the Wrec/W2 matmuls wait on the spikes; the
carry diagonals run off the critical path.  The +-1 encoding of chunk 3
is folded into host-side constants (a ones row in x for d1, a ones
plane in the spike tile for d2), and the t=0 state contribution is a
constant folded into a t=0-only x row so step 0 waits only for x and
W1.  Layer 2 runs one step delayed so the in-order PE stream never
stalls on layer-2 state.

All matmuls are fp8e4 DoubleRow except the plain diagonal matmuls.
"""

import sys
import numpy as np

sys.path.insert(0, "/opt/trn_rl_repo")

import ml_dtypes

F8 = ml_dtypes.float8_e4m3

# Problem constants (hardcoded per contract)
B, T, N_IN, H1, H2, N_OUT = 1024, 100, 120, 512, 256, 35
N_CORES = 8
BC = B // N_CORES  # 128 batch per core
C1 = H1 // 128     # 4 feature chunks, layer 1 (3 on DVE chain, 1 on ACT)
C2 = H2 // 128     # 2 feature chunks, layer 2
KX = N_IN // 2     # 60 rows per DoubleRow plane of x
KXA = KX + 1       # +1 ones row carrying per-feature constants
LAM = 32.0         # PSUM potential scale
S3 = 256.0         # output stationary scale
EPS = 0.0005       # Sign tie-breaker, strictly inside (0, min subnormal)

_CACHE = {}


def _build(a1, r1, b1, a2, r2, b2, bo):
    import concourse.bacc as bacc
    import concourse.mybir as mybir
    import concourse.tile as tile
    from concourse.alu_op_type import AluOpType

    fp32 = mybir.dt.float32
    fp8 = mybir.dt.float8e4
    A = AluOpType
    DR = mybir.MatmulPerfMode.DoubleRow
    RELU = mybir.ActivationFunctionType.Relu
    SIGN = mybir.ActivationFunctionType.Sign

    C1b = LAM * (a1 - 1.0)   # -1.6
    C2b = LAM * (a2 - 1.0)

    nc = bacc.Bacc()

    x_d = nc.declare_dram_parameter("x", [KXA, T, 2, BC], fp8, isOutput=False)
    w1_d = nc.declare_dram_parameter("w1s", [KXA, 2, C1, 128], fp8, isOutput=False)
    wr_d = nc.declare_dram_parameter("wrecs", [128, 2, 2, C1, 128], fp8, isOutput=False)
    w2_d = nc.declare_dram_parameter("w2s", [128, 3, 2, C2, 128], fp8, isOutput=False)
    wdg_d = nc.declare_dram_parameter("wdiags", [128, 4, 128], fp8, isOutput=False)
    w3_d = nc.declare_dram_parameter("w3s", [128, T, 2, 64], fp8, isOutput=False)
    out_d = nc.declare_dram_parameter("out", [N_OUT, BC], fp32, isOutput=True)

    XCH = 10  # x preload chunks
    TP = T // XCH

    with tile.TileContext(nc) as tc:
        with (
            tc.tile_pool(name="wpool", bufs=1) as wpool,
            tc.tile_pool(name="xpool", bufs=1) as xpool,
            tc.tile_pool(name="s1p", bufs=2) as s1p,
            tc.tile_pool(name="gap", bufs=2) as gap,
            tc.tile_pool(name="gbp", bufs=2) as gbp,
            tc.tile_pool(name="s2p", bufs=2) as s2p,
            tc.tile_pool(name="g2p", bufs=2) as g2p,
            tc.tile_pool(name="tmp", bufs=1) as tmp,
            tc.tile_pool(name="psa", bufs=2, space="PSUM") as psa,
            tc.tile_pool(name="psb", bufs=2, space="PSUM") as psb,
            tc.tile_pool(name="ps2", bufs=2, space="PSUM") as ps2,
            tc.tile_pool(name="pso", bufs=1, space="PSUM") as pso,
            tc.tile_pool(name="psw", bufs=1, space="PSUM") as psw,
        ):
            # ---- resident weights; DMA order matters: everything step 0
            # needs goes first, the big w3 table is split and mostly last ----
            w1_s = wpool.tile([KXA, 2, C1, 128], fp8, tag="w1")
            wr_s = wpool.tile([128, 2, 2, C1, 128], fp8, tag="wr")
            w2_s = wpool.tile([128, 3, 2, C2, 128], fp8, tag="w2")
            wdg_s = wpool.tile([128, 4, 128], fp8, tag="wdg")
            w3_s = wpool.tile([128, T, 2, 64], fp8, tag="w3")
            x_tiles = []
            for i in range(XCH):
                xt = xpool.tile([KXA, TP, 2, BC], fp8, tag=f"x{i}")
                x_tiles.append(xt)

            nc.sync.dma_start(wdg_s[:], wdg_d[:])
            nc.scalar.dma_start(w1_s[:], w1_d[:])
            nc.sync.dma_start(x_tiles[0][:], x_d[:, 0:TP, :, :])
            nc.gpsimd.dma_start(wr_s[:], wr_d[:])
            nc.gpsimd.dma_start(w2_s[:], w2_d[:])
            TQ = T // 4
            nc.scalar.dma_start(w3_s[:, 0:TQ], w3_d[:, 0:TQ])
            nc.sync.dma_start(x_tiles[1][:], x_d[:, TP : 2 * TP, :, :])
            biasg1_s = wpool.tile([128, 1], fp32, tag="biasg1")
            nc.vector.memset(biasg1_s[:], -C1b)
            biasg2_s = wpool.tile([128, 1], fp32, tag="biasg2")
            nc.vector.memset(biasg2_s[:], -C2b)
            biassgn_s = wpool.tile([128, 1], fp32, tag="biassgn")
            nc.vector.memset(biassgn_s[:], C1b + EPS)

            # ---- state tiles ----
            # s1ext: planes 0-2 spikes 0/1 (DVE), plane 3 spike +-1 (ACT),
            # plane 4 constant ones, plane 5 constant zeros.  Two manual
            # buffers; only planes 0-3 are rewritten.
            s1_tiles = []
            for i in range(2):
                s1t = s1p.tile([128, 6, BC], fp8, tag=f"s1_{i}")
                nc.vector.memset(s1t[:], 0.0)
                nc.vector.memset(s1t[:, 3, :], -1.0)   # +-1 encoding of s=0
                nc.vector.memset(s1t[:, 4, :], 1.0)
                s1_tiles.append(s1t)
            ga = gap.tile([128, 3, BC], fp8, tag="ga")   # min carry, chunks 0-2
            nc.vector.memset(ga[:], -LAM)
            gb = gbp.tile([128, 1, BC], fp8, tag="gb")   # flipped carry, chunk 3
            nc.vector.memset(gb[:], LAM)
            s2 = s2p.tile([128, C2, BC], fp8, tag="s2")
            nc.vector.memset(s2[:], 0.0)
            g2f = g2p.tile([128, C2, BC], fp8, tag="g2f")
            nc.vector.memset(g2f[:], LAM)

            out_ps = pso.tile([64, BC], fp32, tag="out")

            # x matmuls for step 0
            d1a = psa.tile([128, 3, BC], fp32, tag="d1a")
            d1b = psb.tile([128, 1, BC], fp32, tag="d1b")
            # PE p-state warmup: keep the tensor engine continuously busy
            # through the DMA preamble so real work starts at full clock.
            # Dummy weights are memset locally (no DMA dependency).
            wu_w = wpool.tile([128, 2, 128], fp8, tag="wu_w")
            nc.vector.memset(wu_w[:], 0.0)
            wu_ps = psw.tile([128, 128], fp32, tag="wu")
            for _ in range(30):
                nc.tensor.matmul(wu_ps[:], wu_w[:], wu_w[:, :, 0:BC],
                                 start=True, stop=True, perf_mode=DR,
                                 skip_group_check=True)

            # t=0: all state contributions are constants, folded into the
            # plane-0 ones row of x (active only at t=0), so step 0 needs
            # neither the recurrent weights nor the diagonal stationaries.
            for m in range(3):
                nc.tensor.matmul(d1a[:, m, :], w1_s[:, :, m, :],
                                 x_tiles[0][:, 0, :, :],
                                 start=True, stop=True, perf_mode=DR)
            nc.tensor.matmul(d1b[:, 0, :], w1_s[:, :, 3, :],
                             x_tiles[0][:, 0, :, :],
                             start=True, stop=True, perf_mode=DR)

            s1 = s1_tiles[1]      # state of step -1
            s2_hist = [s2]        # s2_hist[k] = s2(k-1)
            g2_hist = [g2f]

            for t in range(T):
                s1n = s1_tiles[t % 2]

                # ----- close d1(t): B half first, then A half -----
                if t > 0:
                    nc.tensor.matmul(d1a[:, :, :], wdg_s[:, 0, :], ga[:, :, :],
                                     start=False, stop=False,
                                     skip_group_check=True)
                    for kp in (1, 0):
                        for m in range(3):
                            nc.tensor.matmul(
                                d1a[:, m, :], wr_s[:, kp, :, m, :],
                                s1[:, 2 * kp : 2 * kp + 2, :],
                                start=False, stop=(kp == 0),
                                perf_mode=DR, skip_group_check=True,
                            )
                    for kp in (1, 0):
                        nc.tensor.matmul(
                            d1b[:, 0, :], wr_s[:, kp, :, 3, :],
                            s1[:, 2 * kp : 2 * kp + 2, :],
                            start=False, stop=False,
                            perf_mode=DR, skip_group_check=True,
                        )
                    nc.tensor.matmul(d1b[:, :, :], wdg_s[:, 1, :], gb[:, :, :],
                                     start=False, stop=True,
                                     skip_group_check=True)

                # ----- layer-1 carries (sole PSUM readers), then spikes -----
                nc.scalar.activation(
                    s1n[:, 3, :], d1b[:, 0, :], SIGN,
                    bias=biassgn_s[:, 0:1], scale=1.0,
                )
                gb = gbp.tile([128, 1, BC], fp8, tag="gb")
                nc.scalar.activation(
                    gb[:], d1b[:], RELU, bias=biasg1_s[:, 0:1], scale=-1.0
                )
                ga = gap.tile([128, 3, BC], fp8, tag="ga")
                nc.vector.tensor_scalar(
                    ga[:], d1a[:], C1b, 0.0, A.add, A.min
                )
                nc.vector.tensor_scalar(
                    s1n[:, 0:3, :], ga[:], 0.0, None, A.is_ge
                )
                s1 = s1n

                # ----- layer 2 for step t-1 (one step delayed) -----
                if t >= 1:
                    s1m = s1_tiles[(t - 1) % 2]
                    d2 = ps2.tile([128, C2, BC], fp32, tag="d2")
                    for m in range(C2):
                        for kp in range(2):
                            nc.tensor.matmul(
                                d2[:, m, :], w2_s[:, kp, :, m, :],
                                s1m[:, 2 * kp : 2 * kp + 2, :],
                                start=(kp == 0), stop=False,
                                perf_mode=DR, skip_group_check=True,
                            )
                        nc.tensor.matmul(
                            d2[:, m, :], w2_s[:, 2, :, m, :],
                            s1m[:, 4:6, :],
                            start=False, stop=False,
                            perf_mode=DR, skip_group_check=True,
                        )
                    nc.tensor.matmul(d2[:, :, :], wdg_s[:, 2, :],
                                     s2_hist[t - 1][:, :, :],
                                     start=False, stop=False,
                                     skip_group_check=True)
                    nc.tensor.matmul(d2[:, :, :], wdg_s[:, 3, :],
                                     g2_hist[t - 1][:, :, :],
                                     start=False, stop=True,
                                     skip_group_check=True)
                    g2f = g2p.tile([128, C2, BC], fp8, tag="g2f")
                    nc.scalar.activation(
                        g2f[:], d2[:], RELU, bias=biasg2_s[:, 0:1], scale=-1.0
                    )
                    s2 = s2p.tile([128, C2, BC], fp8, tag="s2")
                    nc.vector.tensor_scalar(
                        s2[:], g2f[:], 0.0, None, A.is_le
                    )
                    s2_hist.append(s2)
                    g2_hist.append(g2f)

                # ----- deferred output accumulation for step t-2 -----
                if t >= 2:
                    nc.tensor.matmul(
                        out_ps[:], w3_s[:, t - 2, :, :],
                        s2_hist[t - 1][:, 0:2, :],
                        start=(t == 2), stop=False, perf_mode=DR,
                        skip_group_check=True,
                    )

                # ----- late input/weight chunks, issued from the idle SP
                # sequencer well ahead of first use -----
                if t >= 2 and t % TP == 2 and t // TP + 2 < XCH:
                    i = t // TP + 2
                    nc.sync.dma_start(x_tiles[i][:],
                                      x_d[:, i * TP : (i + 1) * TP, :, :])
                if t in (4, 6, 8):
                    q = (t - 2) // 2
                    nc.sync.dma_start(w3_s[:, q * TQ : (q + 1) * TQ],
                                      w3_d[:, q * TQ : (q + 1) * TQ])

                # ----- x matmuls for step t+1 -----
                if t < T - 1:
                    d1a = psa.tile([128, 3, BC], fp32, tag="d1a")
                    d1b = psb.tile([128, 1, BC], fp32, tag="d1b")
                    xsl = x_tiles[(t + 1) // TP][:, (t + 1) % TP, :, :]
                    for m in range(3):
                        nc.tensor.matmul(d1a[:, m, :], w1_s[:, :, m, :], xsl,
                                         start=True, stop=False, perf_mode=DR)
                    nc.tensor.matmul(d1b[:, 0, :], w1_s[:, :, 3, :], xsl,
                                     start=True, stop=False, perf_mode=DR)

            # ----- epilogue: layer 2 for step T-1, remaining outputs -----
            s1m = s1_tiles[(T - 1) % 2]
            d2 = ps2.tile([128, C2, BC], fp32, tag="d2")
            for m in range(C2):
                for kp in range(2):
                    nc.tensor.matmul(
                        d2[:, m, :], w2_s[:, kp, :, m, :],
                        s1m[:, 2 * kp : 2 * kp + 2, :],
                        start=(kp == 0), stop=False,
                        perf_mode=DR, skip_group_check=True,
                    )
                nc.tensor.matmul(
                    d2[:, m, :], w2_s[:, 2, :, m, :], s1m[:, 4:6, :],
                    start=False, stop=False, perf_mode=DR,
                    skip_group_check=True,
                )
            nc.tensor.matmul(d2[:, :, :], wdg_s[:, 2, :], s2_hist[T - 1][:, :, :],
                             start=False, stop=False, skip_group_check=True)
            nc.tensor.matmul(d2[:, :, :], wdg_s[:, 3, :], g2_hist[T - 1][:, :, :],
                             start=False, stop=True, skip_group_check=True)
            s2 = s2p.tile([128, C2, BC], fp8, tag="s2")
            nc.vector.tensor_scalar(s2[:], d2[:], C2b, 0.0, A.add, A.is_gt)
            s2_hist.append(s2)

            nc.tensor.matmul(
                out_ps[:], w3_s[:, T - 2, :, :], s2_hist[T - 1][:, 0:2, :],
                start=False, stop=False, perf_mode=DR, skip_group_check=True,
            )
            nc.tensor.matmul(
                out_ps[:], w3_s[:, T - 1, :, :], s2_hist[T][:, 0:2, :],
                start=False, stop=True, perf_mode=DR, skip_group_check=True,
            )

            outf = tmp.tile([N_OUT, BC], fp32, tag="outf")
            nc.vector.tensor_copy(outf[:], out_ps[:N_OUT, :])
            nc.sync.dma_start(out_d[:], outf[:])

    nc.compile()
    return nc


def _prep_inputs(x, W1, Wrec, W2, W3, a1, r1, b1, a2, r2, b2, bo):
    cb1 = b1 * (1.0 - r1)
    A1 = -(a1 + cb1)
    cb2 = b2 * (1.0 - r2)
    A2 = -(a2 + cb2)

    W1f = np.asarray(W1, np.float32) * (LAM * (1.0 - a1))    # [H1, N_IN]
    Wrf = np.asarray(Wrec, np.float32) * (LAM * (1.0 - a1))  # [H1, H1]
    Wrf += (LAM * A1) * np.eye(H1, dtype=np.float32)         # fold A1*s1 diag
    W2f = np.asarray(W2, np.float32) * (LAM * (1.0 - a2))    # [H2, H1]
    W3f = np.asarray(W3, np.float32)                          # [N_OUT, H2]

    # chunk 3 (features 384:512) uses +-1 spikes: s = (st+1)/2.  Halve those
    # columns and collect the +1/2 parts as per-feature constants.
    c3 = slice(384, 512)
    corr1 = 0.5 * Wrf[:, c3].sum(axis=1)                     # [H1]
    Wrf[:, c3] *= 0.5
    corr2 = 0.5 * W2f[:, c3].sum(axis=1)                     # [H2]
    W2f[:, c3] *= 0.5

    # w1 stationary [KXA, 2, C1, 128]; ones row (KX, plane 1) carries corr1
    w1s = np.zeros((KXA, 2, C1, 128), np.float32)
    for m in range(C1):
        blk = W1f[m * 128 : (m + 1) * 128]                   # [128, N_IN]
        w1s[:KX, 0, m, :] = blk[:, 0:KX].T
        w1s[:KX, 1, m, :] = blk[:, KX : 2 * KX].T
        w1s[KX, 1, m, :] = corr1[m * 128 : (m + 1) * 128]
        w1s[KX, 0, m, :] = (-corr1[m * 128 : (m + 1) * 128]
                            - a1 * LAM)  # t=0 state constants

    wrs = np.zeros((128, 2, 2, C1, 128), np.float32)
    for m in range(C1):
        blk = Wrf[m * 128 : (m + 1) * 128]                   # [128, H1]
        for kp in range(2):
            for i in range(2):
                k = 2 * kp + i
                wrs[:, kp, i, m, :] = blk[:, k * 128 : (k + 1) * 128].T

    # w2 stationary [128, 3, 2, C2, 128]; pair 2 is (ones, zeros) and its
    # plane-0 row 0 carries corr2.
    w2s = np.zeros((128, 3, 2, C2, 128), np.float32)
    for m in range(C2):
        blk = W2f[m * 128 : (m + 1) * 128]                   # [128, H1]
        for kp in range(2):
            for i in range(2):
                k = 2 * kp + i
                w2s[:, kp, i, m, :] = blk[:, k * 128 : (k + 1) * 128].T
        w2s[0, 2, 0, m, :] = corr2[m * 128 : (m + 1) * 128]

    eye = np.eye(128, dtype=np.float32)
    wga = a1 * eye            # min-form carry, chunks 0-2
    wgb = -a1 * eye           # flipped relu carry, chunk 3
    wds2 = LAM * A2 * eye
    wdg2 = -a2 * eye

    wt = (1.0 - bo ** (T - np.arange(T, dtype=np.float64))) / T
    w3s = np.zeros((128, T, 2, 64), np.float32)
    for t in range(T):
        sc = np.float32(S3 * wt[t])
        w3s[:, t, 0, :N_OUT] = (sc * W3f[:, 0:128]).T
        w3s[:, t, 1, :N_OUT] = (sc * W3f[:, 128:256]).T

    wdiags = np.stack([wga, wgb, wds2, wdg2], axis=1)      # [128, 4, 128]
    shared = dict(
        w1s=w1s.astype(F8), wrecs=wrs.astype(F8), w2s=w2s.astype(F8),
        wdiags=wdiags.astype(F8), w3s=w3s.astype(F8),
    )
    in_maps = []
    for c in range(N_CORES):
        xc = np.asarray(x[c * BC : (c + 1) * BC], np.float32)  # [BC, T, N_IN]
        xfm = xc.transpose(2, 1, 0)                            # [N_IN, T, BC]
        x8 = np.zeros((KXA, T, 2, BC), np.float32)
        x8[:KX, :, 0, :] = xfm[0:KX]
        x8[:KX, :, 1, :] = xfm[KX : 2 * KX]
        x8[KX, :, 1, :] = 1.0
        x8[KX, 0, 0, :] = 1.0   # t=0 indicator row
        in_maps.append(dict(x=np.ascontiguousarray(x8).astype(F8), **shared))
    return in_maps


def kernel(
    x, W1, Wrec, W2, W3,
    alpha1, rho1, beta_a1, alpha2, rho2, beta_a2, beta_out,
    _trace=False,
):
    from concourse.bass_utils import run_bass_kernel_spmd

    sc = [float(np.asarray(v).reshape(-1)[0]) for v in
          (alpha1, rho1, beta_a1, alpha2, rho2, beta_a2, beta_out)]
    if "nc" not in _CACHE:
        _CACHE["nc"] = _build(*sc)
    nc = _CACHE["nc"]

    in_maps = _prep_inputs(x, W1, Wrec, W2, W3, *sc)
    res = run_bass_kernel_spmd(nc, in_maps, list(range(N_CORES)), trace=_trace)

    out = np.empty((B, N_OUT), np.float32)
    for c in range(N_CORES):
        out[c * BC : (c + 1) * BC] = np.asarray(res.results[c]["out"]).T / S3
    if _trace:
        return out, res
    return out


# revision 35
# speedup vs baseline: 1.0065x; 1.0065x over previous
"""Trainium2 Bass kernel for a recurrent adaptive-LIF SNN.

Network (per reference):
    B=1024, T=100, n_in=120, h1=512, h2=256, n_out=35
    per step t:
        cur1 = x_t @ W1.T + s1 @ Wrec.T
        a1' = rho1*a1 + (1-rho1)*s1 ; v1' = alpha1*v1*(1-s1) + (1-alpha1)*cur1
        s1' = (v1' - (1 + beta_a1*a1') > 0)
        cur2 = s1' @ W2.T ; same LIF for layer 2
        vo' = beta_out*vo + (1-beta_out)*(s2' @ W3.T) ; out = mean_t vo

Sharding: data-parallel over batch across 8 cores (BC=128 per core),
weights replicated; the sequential T loop is local per core.

Reformulation (shifted potential): d := LAM*(v - 1 - beta_a*a) lives in
PSUM (LAM=32 keeps fp8 stationaries in normal range).  With C =
LAM*(alpha-1) and A = -(alpha+cb), cb = beta_a*(1-rho):
    spike       s' = (d + C > 0)
    reset carry g' = min(d + C, 0)        [ = LAM * v'(1-s') - shift ]
    d(t+1) = LAM(1-a)*W@in + LAM*A*s' + a*g' + C-as-op-bias
The identity v'(1-s') <-> min(d,0) makes both nonlinear updates UNARY in
d, so no tensor-tensor products are needed.  The residual adaptation
couplings ((alpha-rho)*cb*u and alpha*cb*u*s', coefficients 7.5e-4 /
7.1e-3) are below the fp8 weight-quantization noise and are dropped;
they are exactly zero whenever the layer is silent.
Output integrator is collapsed into per-step weights:
    out = sum_t w_t * W3 @ s2(t),  w_t = (1-beta_out^(T-t))/T,
one DoubleRow matmul per step accumulating into PSUM.

Scheduling: the tile framework serializes readers of the same PSUM tile
(across engines, in emission order), so d1 is split into two PSUM
tiles.  Chunks 0-2 run a DVE chain: min-carry from PSUM, then the spike
derived from the carry in SBUF at 2x.  Chunk 3 runs on ACT: a +-1 Sign
spike directly from PSUM first, then the flipped relu carry as the
serialized second reader (its consumer, the next step's diagonal, has a
full cycle of slack).  Only the Wrec/W2 matmuls wait on the spikes; the
carry diagonals run off the critical path.  The +-1 encoding of chunk 3
is folded into host-side constants (a ones row in x for d1, a ones
plane in the spike tile for d2), and the t=0 state contribution is a
constant folded into a t=0-only x row so step 0 waits only for x and
W1.  Layer 2 runs one step delayed so the in-order PE stream never
stalls on layer-2 state.

All matmuls are fp8e4 DoubleRow except the plain diagonal matmuls.
"""

import sys
import numpy as np

sys.path.insert(0, "/opt/trn_rl_repo")

import ml_dtypes

F8 = ml_dtypes.float8_e4m3

# Problem constants (hardcoded per contract)
B, T, N_IN, H1, H2, N_OUT = 1024, 100, 120, 512, 256, 35
N_CORES = 8
BC = B // N_CORES  # 128 batch per core
C1 = H1 // 128     # 4 feature chunks, layer 1 (3 on DVE chain, 1 on ACT)
C2 = H2 // 128     # 2 feature chunks, layer 2
KX = N_IN // 2     # 60 rows per DoubleRow plane of x
KXA = KX + 1       # +1 ones row carrying per-feature constants
LAM = 32.0         # PSUM potential scale
S3 = 256.0         # output stationary scale
EPS = 0.0005       # Sign tie-breaker, strictly inside (0, min subnormal)

_CACHE = {}


def _build(a1, r1, b1, a2, r2, b2, bo):
    import concourse.bacc as bacc
    import concourse.mybir as mybir
    import concourse.tile as tile
    from concourse.alu_op_type import AluOpType

    fp32 = mybir.dt.float32
    fp8 = mybir.dt.float8e4
    A = AluOpType
    DR = mybir.MatmulPerfMode.DoubleRow
    RELU = mybir.ActivationFunctionType.Relu
    SIGN = mybir.ActivationFunctionType.Sign

    C1b = LAM * (a1 - 1.0)   # -1.6
    C2b = LAM * (a2 - 1.0)

    nc = bacc.Bacc()

    x_d = nc.declare_dram_parameter("x", [KXA, T, 2, BC], fp8, isOutput=False)
    w1_d = nc.declare_dram_parameter("w1s", [KXA, 2, C1, 128], fp8, isOutput=False)
    wr_d = nc.declare_dram_parameter("wrecs", [128, 2, 2, C1, 128], fp8, isOutput=False)
    w2_d = nc.declare_dram_parameter("w2s", [128, 3, 2, C2, 128], fp8, isOutput=False)
    wdg_d = nc.declare_dram_parameter("wdiags", [128, 4, 128], fp8, isOutput=False)
    w3_d = nc.declare_dram_parameter("w3s", [128, T, 2, 64], fp8, isOutput=False)
    out_d = nc.declare_dram_parameter("out", [N_OUT, BC], fp32, isOutput=True)

    XCH = 10  # x preload chunks
    TP = T // XCH

    with tile.TileContext(nc) as tc:
        with (
            tc.tile_pool(name="wpool", bufs=1) as wpool,
            tc.tile_pool(name="xpool", bufs=1) as xpool,
            tc.tile_pool(name="s1p", bufs=2) as s1p,
            tc.tile_pool(name="gap", bufs=2) as gap,
            tc.tile_pool(name="gbp", bufs=2) as gbp,
            tc.tile_pool(name="s2p", bufs=2) as s2p,
            tc.tile_pool(name="g2p", bufs=2) as g2p,
            tc.tile_pool(name="tmp", bufs=1) as tmp,
            tc.tile_pool(name="psa", bufs=2, space="PSUM") as psa,
            tc.tile_pool(name="psb", bufs=2, space="PSUM") as psb,
            tc.tile_pool(name="ps2", bufs=2, space="PSUM") as ps2,
            tc.tile_pool(name="pso", bufs=1, space="PSUM") as pso,
        ):
            # ---- resident weights; DMA order matters: everything step 0
            # needs goes first, the big w3 table is split and mostly last ----
            w1_s = wpool.tile([KXA, 2, C1, 128], fp8, tag="w1")
            wr_s = wpool.tile([128, 2, 2, C1, 128], fp8, tag="wr")
            w2_s = wpool.tile([128, 3, 2, C2, 128], fp8, tag="w2")
            wdg_s = wpool.tile([128, 4, 128], fp8, tag="wdg")
            w3_s = wpool.tile([128, T, 2, 64], fp8, tag="w3")
            x_tiles = []
            for i in range(XCH):
                xt = xpool.tile([KXA, TP, 2, BC], fp8, tag=f"x{i}")
                x_tiles.append(xt)

            nc.sync.dma_start(wdg_s[:], wdg_d[:])
            nc.scalar.dma_start(w1_s[:], w1_d[:])
            nc.sync.dma_start(x_tiles[0][:], x_d[:, 0:TP, :, :])
            nc.gpsimd.dma_start(wr_s[:], wr_d[:])
            nc.gpsimd.dma_start(w2_s[:], w2_d[:])
            TQ = T // 4
            nc.scalar.dma_start(w3_s[:, 0:TQ], w3_d[:, 0:TQ])
            nc.sync.dma_start(x_tiles[1][:], x_d[:, TP : 2 * TP, :, :])
            biasg1_s = wpool.tile([128, 1], fp32, tag="biasg1")
            nc.vector.memset(biasg1_s[:], -C1b)
            biasg2_s = wpool.tile([128, 1], fp32, tag="biasg2")
            nc.vector.memset(biasg2_s[:], -C2b)
            biassgn_s = wpool.tile([128, 1], fp32, tag="biassgn")
            nc.vector.memset(biassgn_s[:], C1b + EPS)

            # ---- state tiles ----
            # s1ext: planes 0-2 spikes 0/1 (DVE), plane 3 spike +-1 (ACT),
            # plane 4 constant ones, plane 5 constant zeros.  Two manual
            # buffers; only planes 0-3 are rewritten.
            s1_tiles = []
            for i in range(2):
                s1t = s1p.tile([128, 6, BC], fp8, tag=f"s1_{i}")
                nc.vector.memset(s1t[:], 0.0)
                nc.vector.memset(s1t[:, 3, :], -1.0)   # +-1 encoding of s=0
                nc.vector.memset(s1t[:, 4, :], 1.0)
                s1_tiles.append(s1t)
            ga = gap.tile([128, 3, BC], fp8, tag="ga")   # min carry, chunks 0-2
            nc.vector.memset(ga[:], -LAM)
            gb = gbp.tile([128, 1, BC], fp8, tag="gb")   # flipped carry, chunk 3
            nc.vector.memset(gb[:], LAM)
            s2 = s2p.tile([128, C2, BC], fp8, tag="s2")
            nc.vector.memset(s2[:], 0.0)
            g2f = g2p.tile([128, C2, BC], fp8, tag="g2f")
            nc.vector.memset(g2f[:], LAM)

            out_ps = pso.tile([64, BC], fp32, tag="out")

            # x matmuls for step 0
            d1a = psa.tile([128, 3, BC], fp32, tag="d1a")
            d1b = psb.tile([128, 1, BC], fp32, tag="d1b")
            # t=0: all state contributions are constants, folded into the
            # plane-0 ones row of x (active only at t=0), so step 0 needs
            # neither the recurrent weights nor the diagonal stationaries.
            for m in range(3):
                nc.tensor.matmul(d1a[:, m, :], w1_s[:, :, m, :],
                                 x_tiles[0][:, 0, :, :],
                                 start=True, stop=True, perf_mode=DR)
            nc.tensor.matmul(d1b[:, 0, :], w1_s[:, :, 3, :],
                             x_tiles[0][:, 0, :, :],
                             start=True, stop=True, perf_mode=DR)

            s1 = s1_tiles[1]      # state of step -1
            s2_hist = [s2]        # s2_hist[k] = s2(k-1)
            g2_hist = [g2f]

            for t in range(T):
                s1n = s1_tiles[t % 2]

                # ----- close d1(t): B half first, then A half -----
                if t > 0:
                    nc.tensor.matmul(d1a[:, :, :], wdg_s[:, 0, :], ga[:, :, :],
                                     start=False, stop=False,
                                     skip_group_check=True)
                    for kp in (1, 0):
                        for m in range(3):
                            nc.tensor.matmul(
                                d1a[:, m, :], wr_s[:, kp, :, m, :],
                                s1[:, 2 * kp : 2 * kp + 2, :],
                                start=False, stop=(kp == 0),
                                perf_mode=DR, skip_group_check=True,
                            )
                    for kp in (1, 0):
                        nc.tensor.matmul(
                            d1b[:, 0, :], wr_s[:, kp, :, 3, :],
                            s1[:, 2 * kp : 2 * kp + 2, :],
                            start=False, stop=False,
                            perf_mode=DR, skip_group_check=True,
                        )
                    nc.tensor.matmul(d1b[:, :, :], wdg_s[:, 1, :], gb[:, :, :],
                                     start=False, stop=True,
                                     skip_group_check=True)

                # ----- layer-1 carries (sole PSUM readers), then spikes -----
                nc.scalar.activation(
                    s1n[:, 3, :], d1b[:, 0, :], SIGN,
                    bias=biassgn_s[:, 0:1], scale=1.0,
                )
                gb = gbp.tile([128, 1, BC], fp8, tag="gb")
                nc.scalar.activation(
                    gb[:], d1b[:], RELU, bias=biasg1_s[:, 0:1], scale=-1.0
                )
                ga = gap.tile([128, 3, BC], fp8, tag="ga")
                nc.vector.tensor_scalar(
                    ga[:], d1a[:], C1b, 0.0, A.add, A.min
                )
                nc.vector.tensor_scalar(
                    s1n[:, 0:3, :], ga[:], 0.0, None, A.is_ge
                )
                s1 = s1n

                # ----- layer 2 for step t-1 (one step delayed) -----
                if t >= 1:
                    s1m = s1_tiles[(t - 1) % 2]
                    d2 = ps2.tile([128, C2, BC], fp32, tag="d2")
                    for m in range(C2):
                        for kp in range(2):
                            nc.tensor.matmul(
                                d2[:, m, :], w2_s[:, kp, :, m, :],
                                s1m[:, 2 * kp : 2 * kp + 2, :],
                                start=(kp == 0), stop=False,
                                perf_mode=DR, skip_group_check=True,
                            )
                        nc.tensor.matmul(
                            d2[:, m, :], w2_s[:, 2, :, m, :],
                            s1m[:, 4:6, :],
                            start=False, stop=False,
                            perf_mode=DR, skip_group_check=True,
                        )
                    nc.tensor.matmul(d2[:, :, :], wdg_s[:, 2, :],
                                     s2_hist[t - 1][:, :, :],
                                     start=False, stop=False,
                                     skip_group_check=True)
                    nc.tensor.matmul(d2[:, :, :], wdg_s[:, 3, :],
                                     g2_hist[t - 1][:, :, :],
                                     start=False, stop=True,
                                     skip_group_check=True)
                    g2f = g2p.tile([128, C2, BC], fp8, tag="g2f")
                    nc.scalar.activation(
                        g2f[:], d2[:], RELU, bias=biasg2_s[:, 0:1], scale=-1.0
                    )
                    s2 = s2p.tile([128, C2, BC], fp8, tag="s2")
                    nc.vector.tensor_scalar(
                        s2[:], g2f[:], 0.0, None, A.is_le
                    )
                    s2_hist.append(s2)
                    g2_hist.append(g2f)

                # ----- deferred output accumulation for step t-2 -----
                if t >= 2:
                    nc.tensor.matmul(
                        out_ps[:], w3_s[:, t - 2, :, :],
                        s2_hist[t - 1][:, 0:2, :],
                        start=(t == 2), stop=False, perf_mode=DR,
                        skip_group_check=True,
                    )

                # ----- late input/weight chunks, issued from the idle SP
                # sequencer well ahead of first use -----
                if t >= 2 and t % TP == 2 and t // TP + 2 < XCH:
                    i = t // TP + 2
                    nc.sync.dma_start(x_tiles[i][:],
                                      x_d[:, i * TP : (i + 1) * TP, :, :])
                if t in (4, 6, 8):
                    q = (t - 2) // 2
                    nc.sync.dma_start(w3_s[:, q * TQ : (q + 1) * TQ],
                                      w3_d[:, q * TQ : (q + 1) * TQ])

                # ----- x matmuls for step t+1 -----
                if t < T - 1:
                    d1a = psa.tile([128, 3, BC], fp32, tag="d1a")
                    d1b = psb.tile([128, 1, BC], fp32, tag="d1b")
                    xsl = x_tiles[(t + 1) // TP][:, (t + 1) % TP, :, :]
                    for m in range(3):
                        nc.tensor.matmul(d1a[:, m, :], w1_s[:, :, m, :], xsl,
                                         start=True, stop=False, perf_mode=DR)
                    nc.tensor.matmul(d1b[:, 0, :], w1_s[:, :, 3, :], xsl,
                                     start=True, stop=False, perf_mode=DR)

            # ----- epilogue: layer 2 for step T-1, remaining outputs -----
            s1m = s1_tiles[(T - 1) % 2]
            d2 = ps2.tile([128, C2, BC], fp32, tag="d2")
            for m in range(C2):
                for kp in range(2):
                    nc.tensor.matmul(
                        d2[:, m, :], w2_s[:, kp, :, m, :],
                        s1m[:, 2 * kp : 2 * kp + 2, :],
                        start=(kp == 0), stop=False,
                        perf_mode=DR, skip_group_check=True,
                    )
                nc.tensor.matmul(
                    d2[:, m, :], w2_s[:, 2, :, m, :], s1m[:, 4:6, :],
                    start=False, stop=False, perf_mode=DR,
                    skip_group_check=True,
                )
            nc.tensor.matmul(d2[:, :, :], wdg_s[:, 2, :], s2_hist[T - 1][:, :, :],
                             start=False, stop=False, skip_group_check=True)
            nc.tensor.matmul(d2[:, :, :], wdg_s[:, 3, :], g2_hist[T - 1][:, :, :],
                             start=False, stop=True, skip_group_check=True)
            s2 = s2p.tile([128, C2, BC], fp8, tag="s2")
            nc.vector.tensor_scalar(s2[:], d2[:], C2b, 0.0, A.add, A.is_gt)
            s2_hist.append(s2)

            nc.tensor.matmul(
                out_ps[:], w3_s[:, T - 2, :, :], s2_hist[T - 1][:, 0:2, :],
                start=False, stop=False, perf_mode=DR, skip_group_check=True,
            )
            nc.tensor.matmul(
                out_ps[:], w3_s[:, T - 1, :, :], s2_hist[T][:, 0:2, :],
                start=False, stop=True, perf_mode=DR, skip_group_check=True,
            )

            outf = tmp.tile([N_OUT, BC], fp32, tag="outf")
            nc.vector.tensor_copy(outf[:], out_ps[:N_OUT, :])
            nc.sync.dma_start(out_d[:], outf[:])

    nc.compile()
    return nc


def _prep_inputs(x, W1, Wrec, W2, W3, a1, r1, b1, a2, r2, b2, bo):
    cb1 = b1 * (1.0 - r1)
    A1 = -(a1 + cb1)
    cb2 = b2 * (1.0 - r2)
    A2 = -(a2 + cb2)

    W1f = np.asarray(W1, np.float32) * (LAM * (1.0 - a1))    # [H1, N_IN]
    Wrf = np.asarray(Wrec, np.float32) * (LAM * (1.0 - a1))  # [H1, H1]
    Wrf += (LAM * A1) * np.eye(H1, dtype=np.float32)         # fold A1*s1 diag
    W2f = np.asarray(W2, np.float32) * (LAM * (1.0 - a2))    # [H2, H1]
    W3f = np.asarray(W3, np.float32)                          # [N_OUT, H2]

    # chunk 3 (features 384:512) uses +-1 spikes: s = (st+1)/2.  Halve those
    # columns and collect the +1/2 parts as per-feature constants.
    c3 = slice(384, 512)
    corr1 = 0.5 * Wrf[:, c3].sum(axis=1)                     # [H1]
    Wrf[:, c3] *= 0.5
    corr2 = 0.5 * W2f[:, c3].sum(axis=1)                     # [H2]
    W2f[:, c3] *= 0.5

    # w1 stationary [KXA, 2, C1, 128]; ones row (KX, plane 1) carries corr1
    w1s = np.zeros((KXA, 2, C1, 128), np.float32)
    for m in range(C1):
        blk = W1f[m * 128 : (m + 1) * 128]                   # [128, N_IN]
        w1s[:KX, 0, m, :] = blk[:, 0:KX].T
        w1s[:KX, 1, m, :] = blk[:, KX : 2 * KX].T
        w1s[KX, 1, m, :] = corr1[m * 128 : (m + 1) * 128]
        w1s[KX, 0, m, :] = (-corr1[m * 128 : (m + 1) * 128]
                            - a1 * LAM)  # t=0 state constants

    wrs = np.zeros((128, 2, 2, C1, 128), np.float32)
    for m in range(C1):
        blk = Wrf[m * 128 : (m + 1) * 128]                   # [128, H1]
        for kp in range(2):
            for i in range(2):
                k = 2 * kp + i
                wrs[:, kp, i, m, :] = blk[:, k * 128 : (k + 1) * 128].T

    # w2 stationary [128, 3, 2, C2, 128]; pair 2 is (ones, zeros) and its
    # plane-0 row 0 carries corr2.
    w2s = np.zeros((128, 3, 2, C2, 128), np.float32)
    for m in range(C2):
        blk = W2f[m * 128 : (m + 1) * 128]                   # [128, H1]
        for kp in range(2):
            for i in range(2):
                k = 2 * kp + i
                w2s[:, kp, i, m, :] = blk[:, k * 128 : (k + 1) * 128].T
        w2s[0, 2, 0, m, :] = corr2[m * 128 : (m + 1) * 128]

    eye = np.eye(128, dtype=np.float32)
    wga = a1 * eye            # min-form carry, chunks 0-2
    wgb = -a1 * eye           # flipped relu carry, chunk 3
    wds2 = LAM * A2 * eye
    wdg2 = -a2 * eye

    wt = (1.0 - bo ** (T - np.arange(T, dtype=np.float64))) / T
    w3s = np.zeros((128, T, 2, 64), np.float32)
    for t in range(T):
        sc = np.float32(S3 * wt[t])
        w3s[:, t, 0, :N_OUT] = (sc * W3f[:, 0:128]).T
        w3s[:, t, 1, :N_OUT] = (sc * W3f[:, 128:256]).T

    wdiags = np.stack([wga, wgb, wds2, wdg2], axis=1)      # [128, 4, 128]
    shared = dict(
        w1s=w1s.astype(F8), wrecs=wrs.astype(F8), w2s=w2s.astype(F8),
        wdiags=wdiags.astype(F8), w3s=w3s.astype(F8),
    )
    in_maps = []
    for c in range(N_CORES):
        xc = np.asarray(x[c * BC : (c + 1) * BC], np.float32)  # [BC, T, N_IN]
        xfm = xc.transpose(2, 1, 0)                            # [N_IN, T, BC]
        x8 = np.zeros((KXA, T, 2, BC), np.float32)
        x8[:KX, :, 0, :] = xfm[0:KX]
        x8[:KX, :, 1, :] = xfm[KX : 2 * KX]
        x8[KX, :, 1, :] = 1.0
        x8[KX, 0, 0, :] = 1.0   # t=0 indicator row
        in_maps.append(dict(x=np.ascontiguousarray(x8).astype(F8), **shared))
    return in_maps


def kernel(
    x, W1, Wrec, W2, W3,
    alpha1, rho1, beta_a1, alpha2, rho2, beta_a2, beta_out,
    _trace=False,
):
    from concourse.bass_utils import run_bass_kernel_spmd

    sc = [float(np.asarray(v).reshape(-1)[0]) for v in
          (alpha1, rho1, beta_a1, alpha2, rho2, beta_a2, beta_out)]
    if "nc" not in _CACHE:
        _CACHE["nc"] = _build(*sc)
    nc = _CACHE["nc"]

    in_maps = _prep_inputs(x, W1, Wrec, W2, W3, *sc)
    res = run_bass_kernel_spmd(nc, in_maps, list(range(N_CORES)), trace=_trace)

    out = np.empty((B, N_OUT), np.float32)
    for c in range(N_CORES):
        out[c * BC : (c + 1) * BC] = np.asarray(res.results[c]["out"]).T / S3
    if _trace:
        return out, res
    return out
